# revision 2
# baseline (speedup 1.0000x reference)
"""DiscriminativeLoss kernel v3 for 8 trn2 NeuronCores (Bass/Tile).

No collectives: core c handles image b = c//2; BOTH cores of a pair compute
the full-image class stats (pass 1) redundantly, then each runs pass 2 on
its own pixel half h = c%2. Removing the mid-kernel AllReduce is worth
~2.3ms/iter in this axon environment.

Pass 1 (full image, 512 DR matmuls): 8 pixel-groups x 256 px per matmul;
one-hot [128,2,80] fp8 stationary (DVE is_equal) x emb [128,2,136] fp8
moving -> psum [80,136] accumulated across all matmuls; 8 diagonal [10,17]
blocks summed on DVE give counts+sums.

Pass 2 (own half, 128 DR matmuls): per chunk ONE matmul [128,2,80] x
[128,2,512] -> psum [80(10k x 8j), 512]. Rows per j-block (22): 16 emb
(-2*c_k stationary), q1+q2 (2-row fp8 split of sum e^2), la/la^2/lb/lb^2
(exact B=64 class mask). u = relu(psum + bias) via DVE/ACT with per-lane
accum, y = sqrt(u + dvar^2) on ACT. Host: final ~500-flop assembly.
"""

import os
import sys

import numpy as np

sys.path.insert(0, "/opt/trn_rl_repo")
os.environ.setdefault("MYCRO_LOCAL_CACHE", "1")

import ml_dtypes  # noqa: E402

BF16 = ml_dtypes.bfloat16
FP8 = ml_dtypes.float8_e4m3

# problem constants (hardcoded per harness contract)
B, E, H, W = 4, 16, 1024, 1024
NIMG = H * W
NCORES = 8
NPIX = NIMG // 2            # pass-2 pixels per core
K = 10
KJ = 80                     # pass-2 lane layout 8k+j
DELTA_VAR = 0.5
DELTA_DST = 1.5
A_W, B_W, R_W = 1.0, 1.0, 0.001
BIGM = 80.0                 # wrong-class mask magnitude; B*ka <= 240 stays
                            # exact in IEEE fp8e4, and B > max dist^2 (62.4)

# pass 1: full image, 8 groups of 256 px (2 DR tiles x 128) per matmul
G1 = 8
E2 = E + 1                  # emb channels + ones column
C1 = G1 * E2                # 136 moving cols per matmul
NMM1 = NIMG // (G1 * 256)   # 512 matmuls
GRM = 8                     # m's per embp DMA

# pass 2: 8 j-blocks of F2 px per chunk; 22 rows per j-block
F2 = 512
NCH2 = NPIX // (8 * F2)     # 128 chunks
RPJ = 22                    # e0..e15, q1, q2, la, la2, lb, lb2
NR2 = 8 * RPJ               # 176 used rows; t0 = rows 0..127, t1 = 128..175
NE1 = 64                    # tile-1 rows shipped (48 used + 16 zero pad,
                            # so the dead-region memset starts 32-aligned)
GRC = 8                     # chunks per e0/e1 DMA

# flat row 22j+r -> (tile, partition) of the 16 emb rows of each j
EROW = [(0, 22 * j) if 22 * j + 15 < 128 else (1, 22 * j - 128)
        for j in range(8)]

_cache = {}


def _consts():
    """Host-side constant arrays shared by all cores."""
    ka = np.arange(K) & 3
    kb = np.arange(K) >> 2
    # kpat2: [128, 2*8*10] bf16: value k at (t, g, k), every partition
    kpat2 = np.broadcast_to(
        np.arange(K, dtype=np.float32), (128, 2, G1, K)).reshape(128, -1)
    # s2base: [128, 2, 80] fp8: q-row ones + aux mask coefs; e-rows zero.
    # fp8e4 here is the IEEE variant (max finite 240), so the linear mask
    # coefficient is B*ka (<=192) against an rhs of 2*la instead of
    # 2*B*ka (384, which would round to Inf) against la.
    s2b = np.zeros((128, 2, KJ), dtype=np.float32)
    aux = np.stack([BIGM * ka, -BIGM * np.ones(K),
                    BIGM * kb, -BIGM * np.ones(K)])  # [4, K]
    lanes = 8 * np.arange(K)
    for j in range(8):
        for r in (16, 17):  # q1, q2 rows: ones
            fr = RPJ * j + r
            s2b[fr % 128, fr // 128, lanes + j] = 1.0
        for a in range(4):
            fr = RPJ * j + 18 + a
            s2b[fr % 128, fr // 128, lanes + j] = aux[a]
    # qsel: [10, 80]: qsel[k, 8k+j] = 1
    qsel = np.zeros((K, KJ), dtype=np.float32)
    for k in range(K):
        qsel[k, 8 * k:8 * k + 8] = 1.0
    # bkh: [80, 1] = -B(ka^2+kb^2) - dvar^2
    bkh = np.zeros((KJ, 1), dtype=np.float32)
    for k in range(K):
        bkh[8 * k:8 * k + 8, 0] = (-BIGM * (ka[k] ** 2 + kb[k] ** 2)
                                   - DELTA_VAR * DELTA_VAR)
    # jcol: [80, 10]: jcol[8k+j, k] = 1
    jcol = np.zeros((KJ, K), dtype=np.float32)
    for k in range(K):
        jcol[8 * k:8 * k + 8, k] = 1.0
    return {
        "kpat2": np.ascontiguousarray(kpat2).astype(BF16),
        "s2base": np.ascontiguousarray(
            s2b.astype(FP8)).reshape(128, 2 * KJ),
        "qsel": qsel,
        "bkh": bkh,
        "jcol": jcol,
        "id10": np.eye(K, dtype=np.float32),
    }


def build_module():
    import concourse.mybir as mybir
    import concourse.tile as tile
    from concourse import bacc

    f32 = mybir.dt.float32
    bf16 = mybir.dt.bfloat16
    fp8 = mybir.dt.float8e4
    Alu = mybir.AluOpType
    Act = mybir.ActivationFunctionType
    DR = mybir.MatmulPerfMode.DoubleRow

    debug = os.environ.get("KV3_DEBUG", "0") == "1"
    nc = bacc.Bacc("TRN2", target_bir_lowering=False, debug=False,
                   num_devices=NCORES)

    # inputs
    embp_d = nc.dram_tensor("embp", [128, NMM1 * 2 * C1], fp8,
                            kind="ExternalInput").ap()
    labp_d = nc.dram_tensor("labp", [128, NMM1 * 2 * G1], bf16,
                            kind="ExternalInput").ap()
    e0_d = nc.dram_tensor("e0", [128, NCH2 * F2], fp8,
                          kind="ExternalInput").ap()
    e1_d = nc.dram_tensor("e1", [NE1, NCH2 * F2], fp8,
                          kind="ExternalInput").ap()
    kpat2_d = nc.dram_tensor("kpat2", [128, 2 * G1 * K], bf16,
                             kind="ExternalInput").ap()
    s2base_d = nc.dram_tensor("s2base", [128, 2 * KJ], fp8,
                              kind="ExternalInput").ap()
    qsel_d = nc.dram_tensor("qsel", [K, KJ], f32, kind="ExternalInput").ap()
    bkh_d = nc.dram_tensor("bkh", [KJ, 1], f32, kind="ExternalInput").ap()
    jcol_d = nc.dram_tensor("jcol", [KJ, K], f32, kind="ExternalInput").ap()
    id10_d = nc.dram_tensor("id10", [K, K], f32, kind="ExternalInput").ap()

    # outputs
    stats_d = nc.dram_tensor("stats", [K, E2], f32, kind="ExternalOutput").ap()
    hpart_d = nc.dram_tensor("hpart", [1, K], f32, kind="ExternalOutput").ap()
    if debug:
        dbgs_d = nc.dram_tensor("dbgs", [KJ, C1], f32,
                                kind="ExternalOutput").ap()
        dbgu_d = nc.dram_tensor("dbgu", [KJ, F2], f32,
                                kind="ExternalOutput").ap()

    with tile.TileContext(nc) as tc:
        with (
            tc.tile_pool(name="consts", bufs=1) as cp,
            tc.tile_pool(name="p1", bufs=3) as p1,
            tc.tile_pool(name="p2", bufs=3) as p2,
            tc.tile_pool(name="ps2", bufs=4, space="PSUM") as psp,
            tc.tile_pool(name="ps1", bufs=1, space="PSUM") as ps1,
        ):
            # ---- small consts (SP queue) ----
            kpat2_t = cp.tile([128, 2 * G1 * K], bf16)
            nc.sync.dma_start(kpat2_t[:], kpat2_d[:])
            s_t = cp.tile([128, 2 * KJ], fp8)
            nc.sync.dma_start(s_t[:], s2base_d[:])
            qsel_t = cp.tile([K, KJ], f32)
            nc.sync.dma_start(qsel_t[:], qsel_d[:])
            bkh_t = cp.tile([KJ, 1], f32)
            nc.sync.dma_start(bkh_t[:], bkh_d[:])
            jcol_t = cp.tile([KJ, K], f32)
            nc.sync.dma_start(jcol_t[:], jcol_d[:])
            id10_t = cp.tile([K, K], f32)
            nc.sync.dma_start(id10_t[:], id10_d[:])
            dv2 = cp.tile([KJ, 1], f32)
            nc.vector.memset(dv2[:], DELTA_VAR * DELTA_VAR)
            zer_t = cp.tile([KJ, F2], bf16)
            nc.vector.memset(zer_t[:], 0.0)

            kpat2_v = kpat2_t[:].rearrange("p (t g k) -> p t g k", t=2, g=G1)
            s_v = s_t[:].rearrange("p (t l) -> p t l", t=2)

            # ---- pass-2 rhs: persistent [128, c, t, f]; zero the dead
            # tile-1 partitions once; stream e0/e1 on the SP queue so the
            # bulk overlaps pass 1 (issued before anything that waits on
            # pass-1 results can block the queue)
            e01_t = cp.tile([128, NCH2 * 2 * F2], fp8)
            e01_v = e01_t[:].rearrange("p (c t f) -> p c t f", t=2, f=F2)
            for i in range(8):
                nc.vector.memset(
                    e01_v[NE1:128, 16 * i:16 * (i + 1), 1, :], 0.0)
            e0_r = e0_d.rearrange("p (c f) -> p c f", f=F2)
            e1_r = e1_d.rearrange("p (c f) -> p c f", f=F2)
            for i in range(NCH2 // GRC):
                cs = slice(GRC * i, GRC * (i + 1))
                nc.sync.dma_start(e01_v[:, cs, 0, :], e0_r[:, cs, :])
                nc.sync.dma_start(e01_v[0:NE1, cs, 1, :], e1_r[:, cs, :])

            # ---- pass 1: full-image stats (gpsimd DMA queue) ----
            lab_t = cp.tile([128, NMM1 * 2 * G1], bf16)
            lab_v = lab_t[:].rearrange("p (m t g) -> p m t g", t=2, g=G1)
            nlc = NMM1 * 2 * G1  # 8192 bf16 cols
            for i in range(8):
                nc.gpsimd.dma_start(
                    lab_t[:, (nlc // 8) * i:(nlc // 8) * (i + 1)],
                    labp_d[:, (nlc // 8) * i:(nlc // 8) * (i + 1)])

            sums_ps = ps1.tile([KJ, C1], f32, tag="ps_a")
            embp_r = embp_d.rearrange("p (a x) -> a p x", a=NMM1 // GRM)
            for mg in range(NMM1 // GRM):
                embc = p1.tile([128, GRM * 2 * C1], fp8, tag="embc")
                nc.gpsimd.dma_start(embc[:], embp_r[mg])
                embc_v = embc[:].rearrange("p (g t x) -> p g t x",
                                           g=GRM, t=2)
                for mi in range(GRM):
                    m = mg * GRM + mi
                    oh = p1.tile([128, 2 * G1 * K], fp8, tag="oh")
                    oh4 = oh[:].rearrange("p (t g k) -> p t g k",
                                          t=2, g=G1)
                    nc.vector.tensor_tensor(
                        out=oh4,
                        in0=lab_v[:, m].unsqueeze(3)
                        .to_broadcast([128, 2, G1, K]),
                        in1=kpat2_v,
                        op=Alu.is_equal)
                    nc.tensor.matmul(
                        sums_ps[:],
                        lhsT=oh[:].rearrange("p (t l) -> p t l", t=2),
                        rhs=embc_v[:, mi],
                        perf_mode=DR,
                        start=(m == 0), stop=(m == NMM1 - 1))

            # ---- extract stats: sum the 8 diagonal [10,17] blocks.
            # DVE partition offsets must be 32-aligned, so gather the
            # blocks with small DMAs, then tree-add at partition 0.
            sums_sb = cp.tile([KJ, C1], f32)
            nc.scalar.copy(sums_sb[:], sums_ps[:])
            if debug:
                nc.sync.dma_start(dbgs_d[:], sums_sb[:])
            sums_g = cp.tile([K, 8, E2], f32)
            for g in range(8):
                nc.gpsimd.dma_start(
                    sums_g[:, g, :],
                    sums_sb[10 * g:10 * g + 10, 17 * g:17 * g + 17])
            pair1 = cp.tile([K, 4, E2], f32)
            nc.vector.tensor_tensor(pair1[:], sums_g[:, 0:4, :],
                                    sums_g[:, 4:8, :], op=Alu.add)
            pair2 = cp.tile([K, 2, E2], f32)
            nc.vector.tensor_tensor(pair2[:], pair1[:, 0:2, :],
                                    pair1[:, 2:4, :], op=Alu.add)
            stats_blk = cp.tile([K, E2], f32)
            nc.vector.tensor_tensor(stats_blk[:], pair2[:, 0, :],
                                    pair2[:, 1, :], op=Alu.add)
            nc.gpsimd.dma_start(stats_d[:], stats_blk[:])

            # ---- centers & pass-2 stationary/bias ----
            cnt_safe = cp.tile([K, 1], f32)
            nc.vector.tensor_scalar(out=cnt_safe[:], in0=stats_blk[:, E:E2],
                                    scalar1=1.0, scalar2=None, op0=Alu.max)
            rec = cp.tile([K, 1], f32)
            nc.vector.reciprocal(rec[:], cnt_safe[:])
            cmat = cp.tile([K, E], f32)
            nc.vector.tensor_scalar(out=cmat[:], in0=stats_blk[:, 0:E],
                                    scalar1=rec[:, 0:1], scalar2=None,
                                    op0=Alu.mult)
            csq = cp.tile([K, E], f32)
            nc.vector.tensor_tensor(csq[:], cmat[:], cmat[:], op=Alu.mult)
            qv = cp.tile([K, 1], f32)
            nc.vector.tensor_reduce(qv[:], csq[:], mybir.AxisListType.X,
                                    Alu.add)

            ct_ps = ps1.tile([E, K], f32, tag="ps_b")
            nc.tensor.matmul(ct_ps[:], lhsT=cmat[:], rhs=id10_t[:],
                             start=True, stop=True)
            ctbm = cp.tile([E, K], fp8)
            nc.scalar.activation(ctbm[:], ct_ps[:], Act.Copy, bias=0.0,
                                 scale=-2.0)
            for j in range(8):
                t_j, p0 = EROW[j]
                sv = s_v[:, t_j, :].rearrange("p (k j) -> p j k", j=8)
                if p0 % 32 == 0:  # DVE needs 32-aligned partition offsets
                    nc.vector.tensor_scalar(
                        out=sv[p0:p0 + E, j, :], in0=ctbm[:],
                        scalar1=0.0, scalar2=None, op0=Alu.add)
                else:
                    nc.gpsimd.dma_start(sv[p0:p0 + E, j, :], ctbm[:])

            qb_ps = ps1.tile([KJ, 1], f32, tag="ps_b")
            nc.tensor.matmul(qb_ps[:], lhsT=qsel_t[:], rhs=qv[:],
                             start=True, stop=True)
            qb2 = cp.tile([KJ, 1], f32)
            nc.scalar.activation(qb2[:], qb_ps[:], Act.Identity,
                                 bias=bkh_t[:, 0:1], scale=1.0)

            # ---- pass 2 ----
            uaccV = cp.tile([KJ, NCH2], f32)
            uaccA = cp.tile([KJ, NCH2 // 8 + 1], f32)
            yacc = cp.tile([KJ, NCH2 // 4], f32)
            nc.vector.memset(uaccV[:], 0.0)
            nc.vector.memset(uaccA[:], 0.0)
            tr_t = cp.tile([KJ, 4 * F2], bf16)
            s2mm = s_t[:].rearrange("p (t l) -> p t l", t=2)
            for c in range(NCH2):
                ps2 = psp.tile([KJ, F2], f32, tag="ps2")
                nc.tensor.matmul(ps2[:], lhsT=s2mm, rhs=e01_v[:, c],
                                 perf_mode=DR, start=True, stop=True)
                if c % 4 == 0:
                    u4 = p2.tile([KJ, 4 * F2], bf16, tag="u4")
                usl = u4[:, (c % 4) * F2:(c % 4 + 1) * F2]
                if c % 8 == 4:
                    nc.scalar.activation(
                        usl, ps2[:], Act.Relu, bias=qb2[:, 0:1], scale=1.0,
                        accum_out=uaccA[:, c // 8:c // 8 + 1])
                else:
                    nc.vector.scalar_tensor_tensor(
                        out=usl, in0=ps2[:], scalar=qb2[:, 0:1],
                        in1=zer_t[:], op0=Alu.add, op1=Alu.max,
                        accum_out=uaccV[:, c:c + 1])
                if c % 4 == 3:
                    nc.scalar.activation(
                        tr_t[:], u4[:], Act.Sqrt, bias=dv2[:, 0:1],
                        scale=1.0, accum_out=yacc[:, c // 4:c // 4 + 1])
                if debug and c == 0:
                    dbgu_t = cp.tile([KJ, F2], f32)
                    nc.vector.scalar_tensor_tensor(
                        out=dbgu_t[:], in0=ps2[:], scalar=qb2[:, 0:1],
                        in1=zer_t[:], op0=Alu.add, op1=Alu.max)
                    nc.sync.dma_start(dbgu_d[:], dbgu_t[:])

            # ---- H assembly: hp = sum(u) - 2d*sum(y) + 2d^2*npp ----
            u1a = cp.tile([KJ, 1], f32)
            u1b = cp.tile([KJ, 1], f32)
            y1 = cp.tile([KJ, 1], f32)
            nc.vector.tensor_reduce(u1a[:], uaccV[:], mybir.AxisListType.X,
                                    Alu.add)
            nc.vector.tensor_reduce(u1b[:], uaccA[:], mybir.AxisListType.X,
                                    Alu.add)
            nc.vector.tensor_reduce(y1[:], yacc[:], mybir.AxisListType.X,
                                    Alu.add)
            u1 = cp.tile([KJ, 1], f32)
            nc.vector.tensor_tensor(u1[:], u1a[:], u1b[:], op=Alu.add)
            hp = cp.tile([KJ, 1], f32)
            nc.vector.scalar_tensor_tensor(
                out=hp[:], in0=y1[:], scalar=-2.0 * DELTA_VAR, in1=u1[:],
                op0=Alu.mult, op1=Alu.add)
            npp = float(F2 * NCH2)
            hp2 = cp.tile([KJ, 1], f32)
            nc.vector.tensor_scalar(
                out=hp2[:], in0=hp[:],
                scalar1=2.0 * DELTA_VAR * DELTA_VAR * npp,
                scalar2=None, op0=Alu.add)
            h_ps = ps1.tile([1, K], f32, tag="ps_a")
            nc.tensor.matmul(h_ps[:], lhsT=hp2[:], rhs=jcol_t[:],
                             start=True, stop=True)
            h_sb = cp.tile([1, K], f32)
            nc.scalar.copy(h_sb[:], h_ps[:])
            nc.gpsimd.dma_start(hpart_d[:], h_sb[:])

    nc.compile()
    return nc


def _prep_image(emb_img, lab_img):
    """Full-image pass-1 buffers. emb_img: [E, NIMG] f32, lab_img: [NIMG]."""
    v = emb_img.reshape(E, NMM1, 2, G1, 128)
    arr = np.empty((128, NMM1, 2, G1, E2), dtype=FP8)
    arr[..., :E] = v.transpose(4, 1, 2, 3, 0).astype(FP8)
    arr[..., E] = np.float32(1.0)
    labp = lab_img.reshape(NMM1, 2, G1, 128).transpose(3, 0, 1, 2)
    return {
        "embp": np.ascontiguousarray(arr).reshape(128, NMM1 * 2 * C1),
        "labp": np.ascontiguousarray(
            labp.astype(np.float32)).astype(BF16).reshape(
                128, NMM1 * 2 * G1),
    }


def _prep_half(esh, lab):
    """Pass-2 rhs rows. esh: [E, NPIX] f32, lab: [NPIX] int."""
    v = esh.reshape(E, NCH2, 8, F2)                       # e c j f
    q = (esh * esh).sum(0)                                # [NPIX] f32
    q1 = q.astype(FP8)
    q2 = (q - q1.astype(np.float32)).astype(FP8)
    labj = lab.reshape(NCH2, 8, F2).astype(np.int32)
    la, lb = labj & 3, labj >> 2
    rows = np.empty((8, RPJ, NCH2, F2), dtype=FP8)
    rows[:, 0:E] = v.transpose(2, 0, 1, 3).astype(FP8)    # j e c f
    rows[:, E] = q1.reshape(NCH2, 8, F2).transpose(1, 0, 2)
    rows[:, E + 1] = q2.reshape(NCH2, 8, F2).transpose(1, 0, 2)
    rows[:, E + 2] = (2 * la).transpose(1, 0, 2).astype(FP8)
    rows[:, E + 3] = (la * la).transpose(1, 0, 2).astype(FP8)
    rows[:, E + 4] = (2 * lb).transpose(1, 0, 2).astype(FP8)
    rows[:, E + 5] = (lb * lb).transpose(1, 0, 2).astype(FP8)
    flat = rows.reshape(NR2, NCH2, F2)                    # row = 22j + r
    e1 = np.zeros((NE1, NCH2, F2), dtype=FP8)
    e1[:NR2 - 128] = flat[128:]
    return {
        "e0": np.ascontiguousarray(flat[:128]).reshape(128, NCH2 * F2),
        "e1": e1.reshape(NE1, NCH2 * F2),
    }


def prepare(embedding, ins_label):
    key = "mod"
    if key not in _cache:
        _cache[key] = build_module()
    nc = _cache[key]

    consts = _consts()
    emb_r = np.asarray(embedding, dtype=np.float32).reshape(B, E, NIMG)
    lab_r = np.asarray(ins_label).reshape(B, NIMG)

    img_maps = [_prep_image(emb_r[b], lab_r[b]) for b in range(B)]
    in_maps = []
    for c in range(NCORES):
        b, h = c // 2, c % 2
        sl = slice(h * NPIX, (h + 1) * NPIX)
        m = dict(consts)
        m.update(img_maps[b])
        m.update(_prep_half(np.ascontiguousarray(emb_r[b, :, sl]),
                            lab_r[b, sl]))
        in_maps.append(m)
    return nc, in_maps


def _host_finalize(stats, hsum):
    """stats: [B, 10, 17]; hsum: [B, 10] summed hinge partials."""
    lv_l, ld_l, lr_l, valid_l = [], [], [], []
    ids = np.arange(K)
    for b in range(B):
        counts = stats[b, :, 16].astype(np.float64)
        sums = stats[b, :, 0:16].astype(np.float64)
        present = (counts > 0) & (ids > 0)
        presf = present.astype(np.float64)
        safe = np.where(counts > 0, counts, 1.0)
        centers = sums / safe[:, None]
        per_inst = hsum[b].astype(np.float64) / safe
        n_inst = presf.sum()
        lv = float((per_inst * presf).sum() / max(n_inst, 1.0))
        cdiff = centers[:, None, :] - centers[None, :, :]
        csq = (cdiff * cdiff).sum(-1)
        pm = present[:, None] & present[None, :] & (ids[:, None] < ids[None, :])
        cdist = np.sqrt(np.where(pm, csq, 1.0))
        ph = np.square(np.maximum(2.0 * DELTA_DST - cdist, 0.0)) * pm
        n_pairs = pm.sum()
        ld = float(ph.sum() / max(n_pairs, 1.0))
        cn = np.sqrt(np.where(present, (centers * centers).sum(-1), 1.0))
        lr = float((cn * presf).sum() / max(n_inst, 1.0))
        valid = 1.0 if n_inst > 0 else 0.0
        lv_l.append(lv * valid)
        ld_l.append(ld * valid)
        lr_l.append(lr * valid)
        valid_l.append(valid)
    vb = max(sum(valid_l), 1.0)
    loss_var = sum(lv_l) / vb
    loss_dst = sum(ld_l) / vb
    loss_reg = sum(lr_l) / vb
    total = A_W * loss_var + B_W * loss_dst + R_W * loss_reg
    return (
        np.float32(total),
        np.float32(loss_var),
        np.float32(loss_dst),
        np.float32(loss_reg),
    )


def kernel(embedding, ins_label):
    from concourse.bass_utils import run_bass_kernel_spmd

    nc, in_maps = prepare(embedding, ins_label)
    res = run_bass_kernel_spmd(nc, in_maps, core_ids=list(range(NCORES)))
    stats = np.stack([res.results[2 * b]["stats"] for b in range(B)])
    hsum = np.zeros((B, K), dtype=np.float64)
    for c in range(NCORES):
        hsum[c // 2] += res.results[c]["hpart"].astype(np.float64).reshape(K)
    return _host_finalize(stats.astype(np.float64), hsum)


if __name__ == "__main__":
    build_module()
    print("build ok")
